# revision 28
# baseline (speedup 1.0000x reference)
"""MoE layer (8 experts, top-2 sigmoid routing, SwiGLU experts + shared expert)
on 8 TRN2 NeuronCores.

Strategy (expert-parallel, host-side token dispatch):
  - Router (sigmoid(x @ gate_w.T), top-2, weight normalization) is tiny
    (~50 MFLOP) and runs on the host; it determines the all-to-all dispatch.
  - Core c owns expert c: it gets the tokens routed to expert c (gathered and
    zero-padded to a common capacity M_pad) plus expert c's Wi/Wo.
  - The shared expert is data-parallel: core c also processes tokens
    [c*512, (c+1)*512) with the (replicated) shared weights.
  - Device kernel computes the two SwiGLU MLP passes in bf16 (fp32 PSUM
    accumulation), feature-major layout (features on partitions, tokens on the
    free dim) so no on-device transposes are needed.
  - Host combine: out[t] = shared_out[t] + sum_e cw[e,t] * expert_out[e][t]
    (the combine weights are applied on the host during the scatter-add).
"""

from contextlib import ExitStack

import ml_dtypes
import numpy as np

import concourse.tile as tile
from concourse import bacc, mybir
from concourse.bass_utils import run_bass_kernel_spmd

E, TOPK, H, I = 8, 2, 768, 1152
I2 = 2 * I
T = 4096
N_CORES = 8
TS = T // N_CORES  # shared-expert tokens per core
P = 128
KH = H // P    # 6 contraction tiles over H
KI = I // P    # 9 contraction tiles over I
BF16 = mybir.dt.bfloat16
F32 = mybir.dt.float32
MAXN = 512     # max tokens per matmul chunk (one fp32 PSUM bank)

_BUILD_CACHE: dict = {}
LAST_RESULTS = None  # BassKernelResults of the most recent device run
USE_SILU = True  # native ACT Silu on HW; set False for CoreSim (not implemented there)


def _ensure_axon_ntff_hook():
    """This image's `antenv` lacks the `axon_hooks` module that
    run_bass_kernel_spmd imports when NTFF tracing is requested (BASS_TRACE=1).
    Install an equivalent shim so profiling works instead of crashing."""
    try:
        import antenv.axon_hooks  # noqa: F401
        return
    except ImportError:
        pass
    import sys
    import types
    try:
        import antenv
    except ImportError:
        return
    mod = types.ModuleType("antenv.axon_hooks")
    holder = {"hook": None}
    mod.set_axon_ntff_profile_hook = lambda h: holder.__setitem__("hook", h)
    mod.get_axon_ntff_profile_hook = lambda: holder["hook"]
    sys.modules["antenv.axon_hooks"] = mod
    antenv.axon_hooks = mod
    so_path = "/opt/axon/libaxon_pjrt.so"
    try:
        import os
        if os.path.exists(so_path):
            from trn_agent_boot.trn_boot import _ntff_profile_via_ctypes
            hook = _ntff_profile_via_ctypes(so_path)
            if hook is not None:
                mod.set_axon_ntff_profile_hook(hook)
    except Exception:
        pass  # hook stays None; bass_utils logs a warning and skips tracing


def _chunk_sizes(m: int) -> list[int]:
    """Split m into ceil(m/512) near-equal chunks, smallest last."""
    n = -(-m // MAXN)
    base, rem = divmod(m, n)
    return [base + 1] * rem + [base] * (n - rem)


def _build(m_pad: int):
    nc = bacc.Bacc("TRN2", target_bir_lowering=False, debug=False,
                   num_devices=N_CORES)

    FI2 = I2 // P  # 18 f-tiles of the Wi output
    xe = nc.dram_tensor("xe", [H, m_pad], BF16, kind="ExternalInput").ap()
    wi = nc.dram_tensor("wi", [H, I2], BF16, kind="ExternalInput").ap()
    wo = nc.dram_tensor("wo", [I, H], BF16, kind="ExternalInput").ap()
    xs = nc.dram_tensor("xs", [H, TS], BF16, kind="ExternalInput").ap()
    # swi arrives host-pre-tiled: swi[ft, p, kt*P+c] = shared_Wi.T[kt*P+p, ft*P+c]
    # so each 128-wide f-tile is one contiguous DMA, loadable in the exact
    # order the PE consumes them during the (DMA-bound) kernel lead-in.
    swi = nc.dram_tensor("swi", [FI2, P, H], BF16, kind="ExternalInput").ap()
    swo = nc.dram_tensor("swo", [I, H], BF16, kind="ExternalInput").ap()
    ye = nc.dram_tensor("ye", [H, m_pad], F32, kind="ExternalOutput").ap()
    ys = nc.dram_tensor("ys", [H, TS], F32, kind="ExternalOutput").ap()

    with ExitStack() as ctx:
        tc = ctx.enter_context(tile.TileContext(nc))
        wpool = ctx.enter_context(tc.tile_pool(name="weights", bufs=1))
        apool = ctx.enter_context(tc.tile_pool(name="act", bufs=3))
        spool = ctx.enter_context(tc.tile_pool(name="silu", bufs=4))
        ypool = ctx.enter_context(tc.tile_pool(name="y", bufs=3))
        psum = ctx.enter_context(tc.tile_pool(name="psum", bufs=6, space="PSUM"))
        wpsum = ctx.enter_context(tc.tile_pool(name="wpsum", bufs=1, space="PSUM"))

        # ~24 matmuls on scratch data fill the otherwise-idle PE during the
        # DMA lead-in: the HAM clock gate sees a busy window and un-throttles
        # (4/8 -> 8/8) before the real matmuls start, instead of ~5us after.
        warm_sink = nc.dram_tensor("warm_sink", [P, MAXN], F32).ap()
        warm_sb = wpool.tile([P, MAXN], BF16, tag="warm", name="warm")
        nc.vector.memset(warm_sb[:], 0.0)
        wps = [wpsum.tile([P, MAXN], F32, tag=f"wps{i}", name=f"wps{i}")
               for i in range(2)]
        for i in range(24):
            nc.tensor.matmul(wps[i % 2], lhsT=warm_sb[:, :P], rhs=warm_sb[:],
                             start=True, stop=True)
        warm_out = ypool.tile([P, MAXN], F32, tag="y", name="warm_out")
        nc.vector.tensor_copy(warm_out[:], wps[1])
        nc.gpsimd.dma_start(warm_sink[:], warm_out[:])

        # All resident tensors (x and weights) are loaded as per-k-tile SBUF
        # tiles: dependency tracking is per tile, so a matmul only waits for
        # the one 128-row slice it reads, and compute starts as soon as the
        # first slices land instead of after the whole 13 MB preload.
        def load_rows(dram_ap, ktiles, tag, fsplit=1):
            src = dram_ap.rearrange("(o p) f -> p o f", p=P)
            fw = dram_ap.shape[1] // fsplit
            out = []
            for kt in range(ktiles):
                parts = []
                for h in range(fsplit):
                    t = wpool.tile([P, fw], BF16, tag=f"{tag}{kt}_{h}",
                                   name=f"{tag}{kt}_{h}")
                    nc.sync.dma_start(t[:], src[:, kt, h * fw:(h + 1) * fw])
                    parts.append(t)
                out.append(parts)
            return out

        # shared-expert job first: its x slice and weights are the smallest
        # loads, so the PE starts ~10us earlier and the (larger) expert
        # weight/token streams hide behind the shared job's compute.
        # xs[kt] first, then swi f-tiles in exact consumption order
        # (A0, B0, A1, B1, ...): the PE's k-chains are fed just-in-time.
        xs_src = xs.rearrange("(o p) f -> p o f", p=P)
        xs_t = []
        for kt in range(KH):
            tx = wpool.tile([P, TS], BF16, tag=f"xs{kt}", name=f"xs{kt}")
            nc.sync.dma_start(tx[:], xs_src[:, kt])
            xs_t.append([tx])
        swi_f = [None] * FI2
        for ft in range(KI):
            for f in (ft, KI + ft):
                t = wpool.tile([P, H], BF16, tag=f"swiF{f}", name=f"swiF{f}")
                nc.sync.dma_start(t[:], swi[f])
                swi_f[f] = t

        named = {}  # late-bound tile lists for the expert job

        # accessors: (ft|ht, kt) -> lhsT AP; x: (kt) -> rhs tile
        sh = dict(
            x=lambda kt: xs_t[kt][0],
            wa=lambda ft, kt: swi_f[ft][:, kt * P:(kt + 1) * P],
            wb=lambda ft, kt: swi_f[KI + ft][:, kt * P:(kt + 1) * P],
            wo=lambda ht, kt: named["swo"][kt][0][:, ht * P:(ht + 1) * P],
        )
        ex = dict(
            x=lambda kt: named["xe"][kt][0],
            wa=lambda ft, kt: named["wi"][kt][0][:, ft * P:(ft + 1) * P],
            wb=lambda ft, kt: named["wi"][kt][1][:, ft * P:(ft + 1) * P],
            wo=lambda ht, kt: named["wo"][kt][0][:, ht * P:(ht + 1) * P],
        )

        # (accessors, y_dram, chunk_off, chunk_sz, silu_on_first)
        chunks = []
        for acc, yd, m, sfirst in ((sh, ys, TS, True), (ex, ye, m_pad, False)):
            off = 0
            for sz in _chunk_sizes(m):
                chunks.append((acc, yd, off, sz, sfirst))
                off += sz

        def emit_wi(c):
            acc, yd, off, sz, sfirst = chunks[c]
            act = apool.tile([P, KI, MAXN], BF16, tag="act", name="act")[:, :, :sz]
            for ft in range(KI):
                ps_a = psum.tile([P, MAXN], F32, tag="ps", name="ps_a")[:, :sz]
                for kt in range(KH):
                    nc.tensor.matmul(ps_a, lhsT=acc["wa"](ft, kt),
                                     rhs=acc["x"](kt)[:, off:off + sz],
                                     start=(kt == 0), stop=(kt == KH - 1))
                ps_b = psum.tile([P, MAXN], F32, tag="ps", name="ps_b")[:, :sz]
                for kt in range(KH):
                    nc.tensor.matmul(ps_b, lhsT=acc["wb"](ft, kt),
                                     rhs=acc["x"](kt)[:, off:off + sz],
                                     start=(kt == 0), stop=(kt == KH - 1))
                sl = spool.tile([P, MAXN], F32, tag="silu", name="sl")[:, :sz]
                ps_s, ps_m = (ps_a, ps_b) if sfirst else (ps_b, ps_a)
                if USE_SILU:
                    # act = silu(s) * m: one ACT op + one DVE mul; PSUM banks
                    # are freed one op earlier than the sigmoid+2-mul form
                    nc.scalar.activation(sl, ps_s,
                                         mybir.ActivationFunctionType.Silu)
                    nc.vector.tensor_mul(act[:, ft, :], sl, ps_m)
                else:
                    # CoreSim fallback: silu(s) = s * sigmoid(s)
                    tmp = spool.tile([P, MAXN], F32, tag="silu2",
                                     name="tmp")[:, :sz]
                    nc.scalar.activation(sl, ps_s,
                                         mybir.ActivationFunctionType.Sigmoid)
                    nc.vector.tensor_mul(tmp, sl, ps_s)
                    nc.vector.tensor_mul(act[:, ft, :], tmp, ps_m)
            return act

        def emit_wo(c, act):
            acc, yd, off, sz, sfirst = chunks[c]
            for ht in range(KH):
                ps_y = psum.tile([P, MAXN], F32, tag="ps", name="ps_y")[:, :sz]
                for kt in range(KI):
                    nc.tensor.matmul(ps_y, lhsT=acc["wo"](ht, kt),
                                     rhs=act[:, kt, :],
                                     start=(kt == 0), stop=(kt == KI - 1))
                yt = ypool.tile([P, MAXN], F32, tag="y", name="yt")[:, :sz]
                # copy on the (otherwise idle) Scalar engine so DVE mul
                # throughput isn't what frees PSUM banks; output DMA on the
                # GpSimd SWDGE queue to stay off the input HWDGE stream
                nc.scalar.copy(yt, ps_y)
                nc.gpsimd.dma_start(
                    yd.rearrange("(o p) m -> p o m", p=P)[:, ht, off:off + sz], yt)

        # software pipeline: Wi(c+1) is emitted before Wo(c) so the PE always
        # has independent matmul work while ACT/DVE finish chunk c's SwiGLU.
        # Remaining weight/x loads are emitted at the latest point that still
        # leaves a full compute chunk of DMA lead time.
        n = len(chunks)
        acts = [None] * n
        acts[0] = emit_wi(0)
        named["swo"] = load_rows(swo, KI, "swo")
        named["xe"] = load_rows(xe, KH, "xe")
        named["wi"] = load_rows(wi, KH, "wi", fsplit=2)
        named["wo"] = load_rows(wo, KI, "wo")
        for c in range(1, n):
            acts[c] = emit_wi(c)
            emit_wo(c - 1, acts[c - 1])
        emit_wo(n - 1, acts[-1])

    nc.compile()
    return nc


def _tile_swi(swiT):
    """(H, 2I) -> (18, P, H): f-tile-major contiguous layout for the device."""
    FI2 = I2 // P
    return np.ascontiguousarray(
        swiT.reshape(KH, P, FI2, P).transpose(2, 1, 0, 3).reshape(FI2, P, H))


def _route(x, gate_w, correction_bias):
    logits = 1.0 / (1.0 + np.exp(-(x @ gate_w.T), dtype=np.float32))  # (T, E)
    sel = logits + correction_bias[None, :]
    order = np.argsort(-sel, axis=1, kind="stable")[:, :TOPK]  # ties -> low index
    w = np.take_along_axis(logits, order, axis=1)
    w = (w / w.sum(axis=1, keepdims=True)).astype(np.float32)
    return order, w


def kernel(**inputs) -> np.ndarray:
    x = np.asarray(inputs["x"], np.float32)
    gate_w = np.asarray(inputs["gate_w"], np.float32)
    bias = np.asarray(inputs["correction_bias"], np.float32)
    Wi = np.asarray(inputs["Wi"], np.float32)
    Wo = np.asarray(inputs["Wo"], np.float32)
    shared_Wi = np.asarray(inputs["shared_Wi"], np.float32)
    shared_Wo = np.asarray(inputs["shared_Wo"], np.float32)

    order, w = _route(x, gate_w, bias)

    idx_per_e, cw_per_e = [], []
    for e in range(E):
        mask = order == e  # (T, K)
        tok = mask.any(axis=1)
        rows = np.nonzero(tok)[0]
        kpos = np.argmax(mask[rows], axis=1)
        idx_per_e.append(rows)
        cw_per_e.append(w[rows, kpos].astype(np.float32))

    mx = max(len(r) for r in idx_per_e)
    m_pad = max(64, mx + (mx & 1))  # exact capacity, kept even for alignment

    bf = ml_dtypes.bfloat16
    xT = np.ascontiguousarray(x.T)  # (H, T) f32
    swiT = _tile_swi(shared_Wi.T.astype(bf))             # (18, P, H)
    swoT = np.ascontiguousarray(shared_Wo.T).astype(bf)  # (I, H)

    in_maps = []
    for c in range(N_CORES):
        rows = idx_per_e[c]
        xe = np.zeros((H, m_pad), bf)
        xe[:, :len(rows)] = xT[:, rows].astype(bf)
        in_maps.append({
            "xe": xe,
            "wi": Wi[c].astype(bf),                      # (H, 2I)
            "wo": Wo[c].astype(bf),                      # (I, H)
            "xs": np.ascontiguousarray(
                xT[:, c * TS:(c + 1) * TS]).astype(bf),  # (H, TS)
            "swi": swiT,
            "swo": swoT,
        })

    if m_pad not in _BUILD_CACHE:
        _BUILD_CACHE[m_pad] = _build(m_pad)
    nc = _BUILD_CACHE[m_pad]

    _ensure_axon_ntff_hook()
    res = run_bass_kernel_spmd(nc, in_maps, list(range(N_CORES)))
    global LAST_RESULTS
    LAST_RESULTS = res

    out = np.zeros((T, H), np.float32)
    for c in range(N_CORES):
        r = res.results[c]
        out[c * TS:(c + 1) * TS] += r["ys"].T
        rows = idx_per_e[c]
        if len(rows):
            out[rows] += r["ye"][:, :len(rows)].T * cw_per_e[c][:, None]
    return out


# revision 30
# speedup vs baseline: 1.0643x; 1.0643x over previous
"""MoE layer (8 experts, top-2 sigmoid routing, SwiGLU experts + shared expert)
on 8 TRN2 NeuronCores.

Strategy (expert-parallel, host-side token dispatch):
  - Router (sigmoid(x @ gate_w.T), top-2, weight normalization) is tiny
    (~50 MFLOP) and runs on the host; it determines the all-to-all dispatch.
  - Core c owns expert c: it gets the tokens routed to expert c (gathered and
    zero-padded to a common capacity M_pad) plus expert c's Wi/Wo.
  - The shared expert is data-parallel: core c also processes tokens
    [c*512, (c+1)*512) with the (replicated) shared weights.
  - Device kernel computes the two SwiGLU MLP passes in bf16 (fp32 PSUM
    accumulation), feature-major layout (features on partitions, tokens on the
    free dim) so no on-device transposes are needed.
  - Host combine: out[t] = shared_out[t] + sum_e cw[e,t] * expert_out[e][t]
    (the combine weights are applied on the host during the scatter-add).
"""

from contextlib import ExitStack

import ml_dtypes
import numpy as np

import concourse.tile as tile
from concourse import bacc, mybir
from concourse.bass_utils import run_bass_kernel_spmd

E, TOPK, H, I = 8, 2, 768, 1152
I2 = 2 * I
T = 4096
N_CORES = 8
TS = T // N_CORES  # shared-expert tokens per core
P = 128
KH = H // P    # 6 contraction tiles over H
KI = I // P    # 9 contraction tiles over I
BF16 = mybir.dt.bfloat16
F32 = mybir.dt.float32
MAXN = 512     # max tokens per matmul chunk (one fp32 PSUM bank)

_BUILD_CACHE: dict = {}
LAST_RESULTS = None  # BassKernelResults of the most recent device run
USE_SILU = True  # native ACT Silu on HW; set False for CoreSim (not implemented there)


def _ensure_axon_ntff_hook():
    """This image's `antenv` lacks the `axon_hooks` module that
    run_bass_kernel_spmd imports when NTFF tracing is requested (BASS_TRACE=1).
    Install an equivalent shim so profiling works instead of crashing."""
    try:
        import antenv.axon_hooks  # noqa: F401
        return
    except ImportError:
        pass
    import sys
    import types
    try:
        import antenv
    except ImportError:
        return
    mod = types.ModuleType("antenv.axon_hooks")
    holder = {"hook": None}
    mod.set_axon_ntff_profile_hook = lambda h: holder.__setitem__("hook", h)
    mod.get_axon_ntff_profile_hook = lambda: holder["hook"]
    sys.modules["antenv.axon_hooks"] = mod
    antenv.axon_hooks = mod
    so_path = "/opt/axon/libaxon_pjrt.so"
    try:
        import os
        if os.path.exists(so_path):
            from trn_agent_boot.trn_boot import _ntff_profile_via_ctypes
            hook = _ntff_profile_via_ctypes(so_path)
            if hook is not None:
                mod.set_axon_ntff_profile_hook(hook)
    except Exception:
        pass  # hook stays None; bass_utils logs a warning and skips tracing


def _chunk_sizes(m: int) -> list[int]:
    """Split m into ceil(m/512) near-equal chunks, smallest last."""
    n = -(-m // MAXN)
    base, rem = divmod(m, n)
    return [base + 1] * rem + [base] * (n - rem)


def _build(m_pad: int):
    nc = bacc.Bacc("TRN2", target_bir_lowering=False, debug=False,
                   num_devices=N_CORES)

    FI2 = I2 // P  # 18 f-tiles of the Wi output
    xe = nc.dram_tensor("xe", [H, m_pad], BF16, kind="ExternalInput").ap()
    wi = nc.dram_tensor("wi", [H, I2], BF16, kind="ExternalInput").ap()
    wo = nc.dram_tensor("wo", [I, H], BF16, kind="ExternalInput").ap()
    xs = nc.dram_tensor("xs", [H, TS], BF16, kind="ExternalInput").ap()
    # swi arrives host-pre-tiled: swi[ft, p, kt*P+c] = shared_Wi.T[kt*P+p, ft*P+c]
    # so each 128-wide f-tile is one contiguous DMA, loadable in the exact
    # order the PE consumes them during the (DMA-bound) kernel lead-in.
    swi = nc.dram_tensor("swi", [FI2, P, H], BF16, kind="ExternalInput").ap()
    swo = nc.dram_tensor("swo", [I, H], BF16, kind="ExternalInput").ap()
    ye = nc.dram_tensor("ye", [H, m_pad], F32, kind="ExternalOutput").ap()
    ys = nc.dram_tensor("ys", [H, TS], F32, kind="ExternalOutput").ap()

    with ExitStack() as ctx:
        tc = ctx.enter_context(tile.TileContext(nc))
        wpool = ctx.enter_context(tc.tile_pool(name="weights", bufs=1))
        apool = ctx.enter_context(tc.tile_pool(name="act", bufs=3))
        spool = ctx.enter_context(tc.tile_pool(name="silu", bufs=4))
        ypool = ctx.enter_context(tc.tile_pool(name="y", bufs=3))
        psum = ctx.enter_context(tc.tile_pool(name="psum", bufs=6, space="PSUM"))
        wpsum = ctx.enter_context(tc.tile_pool(name="wpsum", bufs=1, space="PSUM"))

        # ~24 matmuls on scratch data fill the otherwise-idle PE during the
        # DMA lead-in: the HAM clock gate sees a busy window and un-throttles
        # (4/8 -> 8/8) before the real matmuls start, instead of ~5us after.
        warm_sink = nc.dram_tensor("warm_sink", [P, MAXN], F32).ap()
        warm_sb = wpool.tile([P, MAXN], BF16, tag="warm", name="warm")
        nc.vector.memset(warm_sb[:], 0.0)
        wps = [wpsum.tile([P, MAXN], F32, tag=f"wps{i}", name=f"wps{i}")
               for i in range(2)]
        for i in range(8):
            nc.tensor.matmul(wps[i % 2], lhsT=warm_sb[:, :P], rhs=warm_sb[:],
                             start=True, stop=True)
        warm_out = ypool.tile([P, MAXN], F32, tag="y", name="warm_out")
        nc.vector.tensor_copy(warm_out[:], wps[1])
        nc.gpsimd.dma_start(warm_sink[:], warm_out[:])

        # All resident tensors (x and weights) are loaded as per-k-tile SBUF
        # tiles: dependency tracking is per tile, so a matmul only waits for
        # the one 128-row slice it reads, and compute starts as soon as the
        # first slices land instead of after the whole 13 MB preload.
        def load_rows(dram_ap, ktiles, tag, fsplit=1):
            src = dram_ap.rearrange("(o p) f -> p o f", p=P)
            fw = dram_ap.shape[1] // fsplit
            out = []
            for kt in range(ktiles):
                parts = []
                for h in range(fsplit):
                    t = wpool.tile([P, fw], BF16, tag=f"{tag}{kt}_{h}",
                                   name=f"{tag}{kt}_{h}")
                    nc.sync.dma_start(t[:], src[:, kt, h * fw:(h + 1) * fw])
                    parts.append(t)
                out.append(parts)
            return out

        # shared-expert job first: its x slice and weights are the smallest
        # loads, so the PE starts ~10us earlier and the (larger) expert
        # weight/token streams hide behind the shared job's compute.
        # xs[kt] first, then swi f-tiles in exact consumption order
        # (A0, B0, A1, B1, ...): the PE's k-chains are fed just-in-time.
        xs_src = xs.rearrange("(o p) f -> p o f", p=P)
        xs_t = []
        for kt in range(KH):
            tx = wpool.tile([P, TS], BF16, tag=f"xs{kt}", name=f"xs{kt}")
            nc.sync.dma_start(tx[:], xs_src[:, kt])
            xs_t.append([tx])
        swi_f = [None] * FI2
        for ft in range(KI):
            for f in (ft, KI + ft):
                t = wpool.tile([P, H], BF16, tag=f"swiF{f}", name=f"swiF{f}")
                nc.sync.dma_start(t[:], swi[f])
                swi_f[f] = t

        named = {}  # late-bound tile lists for the expert job

        # accessors: (ft|ht, kt) -> lhsT AP; x: (kt) -> rhs tile
        sh = dict(
            x=lambda kt: xs_t[kt][0],
            wa=lambda ft, kt: swi_f[ft][:, kt * P:(kt + 1) * P],
            wb=lambda ft, kt: swi_f[KI + ft][:, kt * P:(kt + 1) * P],
            wo=lambda ht, kt: named["swo"][kt][0][:, ht * P:(ht + 1) * P],
        )
        ex = dict(
            x=lambda kt: named["xe"][kt][0],
            wa=lambda ft, kt: named["wi"][kt][0][:, ft * P:(ft + 1) * P],
            wb=lambda ft, kt: named["wi"][kt][1][:, ft * P:(ft + 1) * P],
            wo=lambda ht, kt: named["wo"][kt][0][:, ht * P:(ht + 1) * P],
        )

        # (accessors, y_dram, chunk_off, chunk_sz, silu_on_first)
        chunks = []
        for acc, yd, m, sfirst in ((sh, ys, TS, True), (ex, ye, m_pad, False)):
            off = 0
            for sz in _chunk_sizes(m):
                chunks.append((acc, yd, off, sz, sfirst))
                off += sz

        def emit_wi(c):
            acc, yd, off, sz, sfirst = chunks[c]
            act = apool.tile([P, KI, MAXN], BF16, tag="act", name="act")[:, :, :sz]
            for ft in range(KI):
                ps_a = psum.tile([P, MAXN], F32, tag="ps", name="ps_a")[:, :sz]
                for kt in range(KH):
                    nc.tensor.matmul(ps_a, lhsT=acc["wa"](ft, kt),
                                     rhs=acc["x"](kt)[:, off:off + sz],
                                     start=(kt == 0), stop=(kt == KH - 1))
                ps_b = psum.tile([P, MAXN], F32, tag="ps", name="ps_b")[:, :sz]
                for kt in range(KH):
                    nc.tensor.matmul(ps_b, lhsT=acc["wb"](ft, kt),
                                     rhs=acc["x"](kt)[:, off:off + sz],
                                     start=(kt == 0), stop=(kt == KH - 1))
                sl = spool.tile([P, MAXN], F32, tag="silu", name="sl")[:, :sz]
                ps_s, ps_m = (ps_a, ps_b) if sfirst else (ps_b, ps_a)
                if USE_SILU:
                    # act = silu(s) * m: one ACT op + one DVE mul; PSUM banks
                    # are freed one op earlier than the sigmoid+2-mul form
                    nc.scalar.activation(sl, ps_s,
                                         mybir.ActivationFunctionType.Silu)
                    nc.vector.tensor_mul(act[:, ft, :], sl, ps_m)
                else:
                    # CoreSim fallback: silu(s) = s * sigmoid(s)
                    tmp = spool.tile([P, MAXN], F32, tag="silu2",
                                     name="tmp")[:, :sz]
                    nc.scalar.activation(sl, ps_s,
                                         mybir.ActivationFunctionType.Sigmoid)
                    nc.vector.tensor_mul(tmp, sl, ps_s)
                    nc.vector.tensor_mul(act[:, ft, :], tmp, ps_m)
            return act

        def emit_wo(c, act):
            acc, yd, off, sz, sfirst = chunks[c]
            for ht in range(KH):
                ps_y = psum.tile([P, MAXN], F32, tag="ps", name="ps_y")[:, :sz]
                for kt in range(KI):
                    nc.tensor.matmul(ps_y, lhsT=acc["wo"](ht, kt),
                                     rhs=act[:, kt, :],
                                     start=(kt == 0), stop=(kt == KI - 1))
                yt = ypool.tile([P, MAXN], F32, tag="y", name="yt")[:, :sz]
                # copy on the (otherwise idle) Scalar engine so DVE mul
                # throughput isn't what frees PSUM banks; output DMA on the
                # GpSimd SWDGE queue to stay off the input HWDGE stream
                nc.scalar.copy(yt, ps_y)
                nc.gpsimd.dma_start(
                    yd.rearrange("(o p) m -> p o m", p=P)[:, ht, off:off + sz], yt)

        # software pipeline: Wi(c+1) is emitted before Wo(c) so the PE always
        # has independent matmul work while ACT/DVE finish chunk c's SwiGLU.
        # Remaining weight/x loads are emitted at the latest point that still
        # leaves a full compute chunk of DMA lead time.
        n = len(chunks)
        acts = [None] * n
        acts[0] = emit_wi(0)
        named["swo"] = load_rows(swo, KI, "swo")
        named["xe"] = load_rows(xe, KH, "xe")
        named["wi"] = load_rows(wi, KH, "wi", fsplit=2)
        named["wo"] = load_rows(wo, KI, "wo")
        # shared Wo BEFORE the first expert Wi: its weights are already
        # resident, so the PE never head-of-line blocks on the expert weight
        # stream (an idle window >3.4us would re-throttle the HAM clock gate)
        emit_wo(0, acts[0])
        if n > 1:
            acts[1] = emit_wi(1)
            for c in range(2, n):
                acts[c] = emit_wi(c)
                emit_wo(c - 1, acts[c - 1])
            emit_wo(n - 1, acts[n - 1])

    nc.compile()
    return nc


def _tile_swi(swiT):
    """(H, 2I) -> (18, P, H): f-tile-major contiguous layout for the device."""
    FI2 = I2 // P
    return np.ascontiguousarray(
        swiT.reshape(KH, P, FI2, P).transpose(2, 1, 0, 3).reshape(FI2, P, H))


def _route(x, gate_w, correction_bias):
    logits = 1.0 / (1.0 + np.exp(-(x @ gate_w.T), dtype=np.float32))  # (T, E)
    sel = logits + correction_bias[None, :]
    order = np.argsort(-sel, axis=1, kind="stable")[:, :TOPK]  # ties -> low index
    w = np.take_along_axis(logits, order, axis=1)
    w = (w / w.sum(axis=1, keepdims=True)).astype(np.float32)
    return order, w


def kernel(**inputs) -> np.ndarray:
    x = np.asarray(inputs["x"], np.float32)
    gate_w = np.asarray(inputs["gate_w"], np.float32)
    bias = np.asarray(inputs["correction_bias"], np.float32)
    Wi = np.asarray(inputs["Wi"], np.float32)
    Wo = np.asarray(inputs["Wo"], np.float32)
    shared_Wi = np.asarray(inputs["shared_Wi"], np.float32)
    shared_Wo = np.asarray(inputs["shared_Wo"], np.float32)

    order, w = _route(x, gate_w, bias)

    idx_per_e, cw_per_e = [], []
    for e in range(E):
        mask = order == e  # (T, K)
        tok = mask.any(axis=1)
        rows = np.nonzero(tok)[0]
        kpos = np.argmax(mask[rows], axis=1)
        idx_per_e.append(rows)
        cw_per_e.append(w[rows, kpos].astype(np.float32))

    mx = max(len(r) for r in idx_per_e)
    m_pad = max(64, mx + (mx & 1))  # exact capacity, kept even for alignment

    bf = ml_dtypes.bfloat16
    xT = np.ascontiguousarray(x.T)  # (H, T) f32
    swiT = _tile_swi(shared_Wi.T.astype(bf))             # (18, P, H)
    swoT = np.ascontiguousarray(shared_Wo.T).astype(bf)  # (I, H)

    in_maps = []
    for c in range(N_CORES):
        rows = idx_per_e[c]
        xe = np.zeros((H, m_pad), bf)
        xe[:, :len(rows)] = xT[:, rows].astype(bf)
        in_maps.append({
            "xe": xe,
            "wi": Wi[c].astype(bf),                      # (H, 2I)
            "wo": Wo[c].astype(bf),                      # (I, H)
            "xs": np.ascontiguousarray(
                xT[:, c * TS:(c + 1) * TS]).astype(bf),  # (H, TS)
            "swi": swiT,
            "swo": swoT,
        })

    if m_pad not in _BUILD_CACHE:
        _BUILD_CACHE[m_pad] = _build(m_pad)
    nc = _BUILD_CACHE[m_pad]

    _ensure_axon_ntff_hook()
    res = run_bass_kernel_spmd(nc, in_maps, list(range(N_CORES)))
    global LAST_RESULTS
    LAST_RESULTS = res

    out = np.zeros((T, H), np.float32)
    for c in range(N_CORES):
        r = res.results[c]
        out[c * TS:(c + 1) * TS] += r["ys"].T
        rows = idx_per_e[c]
        if len(rows):
            out[rows] += r["ye"][:, :len(rows)].T * cw_per_e[c][:, None]
    return out
